# revision 26
# baseline (speedup 1.0000x reference)
"""Chamfer distance kernel for 8 Trainium2 NeuronCores.

Problem: template [4, 8192, 3], source [4, 8192, 3] (fp32)
  d[b,n,m] = ||template[b,n] - source[b,m]||^2
  out[b] = mean_n min_m d + mean_m min_n d            (shape [4], fp32)

Sharding: 8 cores = 4 batches x 2 template-halves. Each core computes its
4096x8192 block of the distance matrix ONCE on the TensorEngine (augmented
K=18 matmul: d = n0 + n1 - 2<t,s>, with bf16 hi/lo coordinate splits so
every product is exact in fp32 PSUM accumulation), and reduces it in both
directions. DVE is the bottleneck at ~2 min-ALU-ops per element (one per
direction) at 2 ops/cycle in bf16 2x mode; everything else is arranged to
keep it gap-free:
  - ScalarE converts each PSUM tile to a bf16 SBUF row-panel (the only
    engine besides DVE that can read PSUM, and it cannot min).
  - col-mins: one wide DVE TT-min accumulate per row tile, partition-
    reduced at the end through PE transposes + DVE segmented reduces,
    with per-round output streaming.
  - row-mins: two-level TT-min halving per tile (8192->2048 into a
    rowtail slot), then batched 4-slot group folds + one 1x reduce per
    4 tiles (minimizes per-op init overhead).
  - ramp: row tiles 0-1 are processed chunk-by-chunk behind the input
    DMA; tiles <=2 run on PE row-group 0 only while the on-chip DMA
    builds the partition-offset-32 input replica that later tiles use
    for LDWEIGHTS/matmul row-group alternation.
Host combines: d01 from row-min sums, d10 from elementwise min of the two
halves' col-min vectors.
"""

import numpy as np
import ml_dtypes

BF = ml_dtypes.bfloat16

B = 4
NPTS = 8192  # template points per batch
MPTS = 8192  # source points per batch
NCORES = 8
NT = NPTS // 2  # template rows per core (half batch)
K = 18  # augmented contraction slots
PTILE = 128  # row tile (PSUM partitions)
CW = 2048  # ScalarE copy width (4 PSUM banks per psum tile)
NCP = MPTS // CW  # 2 copies per row tile
NROW = NT // PTILE  # 32 row tiles
NCOLK = MPTS // PTILE  # 64 columns of colmins output
HALVE_STOP = 2048  # per-tile chain stops here; group folds + reduce finish

_BIG = 3.0e38


def _bf16_parts(x64, n):
    """Split float64 array into n bf16 terms; sum of terms ~= x64."""
    parts = []
    r = np.array(x64, dtype=np.float64, copy=True)
    for _ in range(n):
        p = r.astype(BF)
        parts.append(p)
        r -= p.astype(np.float64)
    return parts


def _prep_core(templ_half, source):
    """Build the [K, NT] and [K, MPTS] bf16 slot matrices for one core.

    Slot layout (template side . source side):
      per coord c: (xh, xh, xl, xl) . (-2yh, -2yl, -2yh, -2yl)   -> 12 slots
      n0 (3-way split) . (1, 1, 1)                                -> 3 slots
      (1, 1, 1) . n1 (3-way split)                                -> 3 slots
    so sum_k ta[k,n]*sa[k,m] = ||t~_n - s~_m||^2 (t~, s~ = 16-bit-split
    coordinates; all bf16 products are exact in fp32 accumulation).
    """
    nt = templ_half.shape[0]
    ms = source.shape[0]
    t = templ_half.astype(np.float64)
    s = source.astype(np.float64)
    ta = np.zeros((K, nt), dtype=BF)
    sa = np.zeros((K, ms), dtype=BF)
    t_eff = np.zeros_like(t)
    s_eff = np.zeros_like(s)
    k = 0
    for c in range(3):
        xh, xl = _bf16_parts(t[:, c], 2)
        yh, yl = _bf16_parts(s[:, c], 2)
        t_eff[:, c] = xh.astype(np.float64) + xl.astype(np.float64)
        s_eff[:, c] = yh.astype(np.float64) + yl.astype(np.float64)
        m2yh = (-2.0 * yh.astype(np.float64)).astype(BF)  # exact (x2 = exp+1)
        m2yl = (-2.0 * yl.astype(np.float64)).astype(BF)
        ta[k + 0], sa[k + 0] = xh, m2yh
        ta[k + 1], sa[k + 1] = xh, m2yl
        ta[k + 2], sa[k + 2] = xl, m2yh
        ta[k + 3], sa[k + 3] = xl, m2yl
        k += 4
    n0 = (t_eff**2).sum(axis=1)
    n1 = (s_eff**2).sum(axis=1)
    ones_t = np.ones(nt, dtype=BF)
    ones_s = np.ones(ms, dtype=BF)
    for part in _bf16_parts(n0, 3):
        ta[k], sa[k] = part, ones_s
        k += 1
    for part in _bf16_parts(n1, 3):
        ta[k], sa[k] = ones_t, part
        k += 1
    assert k == K
    return ta, sa


def _build_bass():
    from contextlib import ExitStack

    import concourse.bacc as bacc
    import concourse.tile as tile
    from concourse import mybir

    f32 = mybir.dt.float32
    bf16 = mybir.dt.bfloat16
    MIN = mybir.AluOpType.min

    nc = bacc.Bacc("TRN2", target_bir_lowering=False)
    ta = nc.dram_tensor("ta", [K, NT], bf16, kind="ExternalInput")
    sa = nc.dram_tensor("sa", [K, MPTS], bf16, kind="ExternalInput")
    ident = nc.dram_tensor("ident", [PTILE, PTILE], bf16, kind="ExternalInput")
    rowmins = nc.dram_tensor("rowmins", [PTILE, NROW], f32, kind="ExternalOutput")
    colmins = nc.dram_tensor("colmins", [PTILE, NCOLK], f32, kind="ExternalOutput")

    with tile.TileContext(nc) as tc, ExitStack() as ctx:
        consts = ctx.enter_context(tc.tile_pool(name="consts", bufs=1))
        accs = ctx.enter_context(tc.tile_pool(name="accs", bufs=1))
        dpool = ctx.enter_context(tc.tile_pool(name="dpool", bufs=4))
        pspool = ctx.enter_context(tc.tile_pool(name="ps", bufs=2, space="PSUM"))

        # Input loads, ordered so row-tile 0 can start as early as possible:
        # tile-0 weights (tiny) first, then the sa column chunks (split
        # between the two HWDGE queues), then the rest of ta. The replica at
        # partition offset 32 (for PE row-group alternation from tile 1 on)
        # is made by on-chip SBUF->SBUF DMA instead of a second HBM load.
        ta_s = consts.tile([32 + K, NT], bf16, name="ta_s", tag="ta_s")
        sa_s = consts.tile([32 + K, MPTS], bf16, name="sa_s", tag="sa_s")
        nc.sync.dma_start(out=ta_s[0:K, 0:PTILE], in_=ta[:, 0:PTILE])
        for c in range(4):
            lsl = slice(c * 2048, c * 2048 + 1024)
            rsl = slice(c * 2048 + 1024, (c + 1) * 2048)
            nc.sync.dma_start(out=sa_s[0:K, lsl], in_=sa[:, lsl])
            nc.scalar.dma_start(out=sa_s[0:K, rsl], in_=sa[:, rsl])
        nc.scalar.dma_start(out=ta_s[0:K, PTILE:], in_=ta[:, PTILE:])
        for c in range(4):
            csl = slice(c * 2048, (c + 1) * 2048)
            nc.gpsimd.dma_start(out=sa_s[32 : 32 + K, csl], in_=sa_s[0:K, csl])
        nc.gpsimd.dma_start(out=ta_s[32 : 32 + K, :], in_=ta_s[0:K, :])
        id_s = consts.tile([PTILE, PTILE], bf16, name="id_s", tag="id_s")
        nc.gpsimd.dma_start(out=id_s, in_=ident[:, :])

        colacc = accs.tile([PTILE, MPTS], bf16, name="colacc", tag="colacc")
        rowtail = accs.tile(
            [PTILE, 4, HALVE_STOP], bf16, name="rowtail", tag="rowtail"
        )
        rowmins_s = accs.tile([PTILE, NROW], f32, name="rowmins_s", tag="rowmins_s")
        colmins_s = accs.tile([PTILE, NCOLK], f32, name="colmins_s", tag="colmins_s")

        for i in range(NROW):
            d = dpool.tile([PTILE, MPTS], bf16, name="d", tag="d")
            for cp in range(NCP):
                ps = pspool.tile([PTILE, CW], f32, name="ps", tag="ps")
                for q in range(CW // 512):
                    col0 = cp * CW + q * 512
                    rg = 0 if i <= 2 else 32 * ((cp * (CW // 512) + q) % 2)
                    nc.tensor.matmul(
                        ps[:, q * 512 : (q + 1) * 512],
                        ta_s[rg : rg + K, i * PTILE : (i + 1) * PTILE],
                        sa_s[rg : rg + K, col0 : col0 + 512],
                        start=True,
                        stop=True,
                        tile_position=(rg, 0),
                    )
                dsl = slice(cp * CW, (cp + 1) * CW)
                nc.scalar.copy(d[:, dsl], ps)
                if i <= 1:
                    # Tiles 0 and 1 are processed chunk-by-chunk so DVE work
                    # starts as soon as each SE chunk copy lands — this hides
                    # the input-DMA + ScalarE pipeline ramp. Chunk row-mins
                    # accumulate into the tile's rowtail slot.
                    if i == 0:
                        nc.vector.tensor_copy(colacc[:, dsl], d[:, dsl])
                    else:
                        nc.vector.tensor_tensor(
                            out=colacc[:, dsl],
                            in0=d[:, dsl],
                            in1=colacc[:, dsl],
                            op=MIN,
                        )
                    if cp == 0:
                        nc.vector.tensor_copy(rowtail[:, i, :], d[:, dsl])
                    else:
                        nc.vector.tensor_tensor(
                            out=rowtail[:, i, :],
                            in0=d[:, dsl],
                            in1=rowtail[:, i, :],
                            op=MIN,
                        )
            if i <= 1:
                continue
            # Column direction: one wide min-accumulate over the row panel.
            nc.vector.tensor_tensor(out=colacc, in0=d, in1=colacc, op=MIN)
            # Row direction, first halving level: fold odd 2048-slices into
            # even ones — both pairs in one 3D-AP TT (stride 2*CW, count 2).
            dv = d.rearrange("p (n c) -> p n c", c=CW)
            nc.vector.tensor_tensor(
                out=dv[:, 0::2, :], in0=dv[:, 0::2, :], in1=dv[:, 1::2, :], op=MIN
            )
            # Second (final) per-tile level: the two live 2048-slices fold
            # straight into the rowtail slot; the rest of the reduction is
            # batched per 4 tiles as 4-slot-wide 2x folds + one 1x reduce.
            nc.vector.tensor_tensor(
                out=rowtail[:, i % 4, :],
                in0=d[:, 0:CW],
                in1=d[:, 2 * CW : 3 * CW],
                op=MIN,
            )
            if i % 4 == 3:
                w = HALVE_STOP // 2
                while w >= 128:
                    nc.vector.tensor_tensor(
                        out=rowtail[:, :, 0:w],
                        in0=rowtail[:, :, 0:w],
                        in1=rowtail[:, :, w : 2 * w],
                        op=MIN,
                    )
                    w //= 2
                nc.vector.tensor_reduce(
                    out=rowmins_s[:, i - 3 : i + 1],
                    in_=rowtail[:, :, 0:128],
                    axis=mybir.AxisListType.X,
                    op=MIN,
                )

        # Row mins are complete after the last group reduce — store them
        # while the endgame runs.
        nc.sync.dma_start(out=rowmins[:, :], in_=rowmins_s)

        # Partition-reduce the column accumulators: PE transpose 128x128
        # blocks into PSUM (as bf16 slices of the fp32 pool tiles, one per
        # 2KB bank), then DVE segmented min-reduce (3D AP, axis X).
        kk = 0
        nper = CW // 512  # transposes per psum tile (one per bank)
        for t0 in range(0, NCOLK, nper):
            ps = pspool.tile([PTILE, CW], f32, name="ps", tag="ps")
            psb = ps.bitcast(bf16)  # [128, 2*CW] bf16 view
            for u in range(nper):
                t = t0 + u  # source block index: points 128*t .. 128*t+127
                nc.tensor.transpose(
                    psb[:, u * 1024 : u * 1024 + PTILE],
                    colacc[:, t * PTILE : (t + 1) * PTILE],
                    id_s,
                )
            seg = psb.rearrange("p (n x) -> p n x", x=1024)[:, :, :PTILE]
            nc.vector.tensor_reduce(
                out=colmins_s[:, kk : kk + nper],
                in_=seg,
                axis=mybir.AxisListType.X,
                op=MIN,
            )
            # Stream this round's colmins out immediately so only the last
            # round's store sits on the tail.
            nc.sync.dma_start(
                out=colmins[:, kk : kk + nper], in_=colmins_s[:, kk : kk + nper]
            )
            kk += nper
        assert kk == NCOLK
    nc.compile()
    return nc


_NC_CACHE = {}


def _get_nc():
    if "nc" not in _NC_CACHE:
        _NC_CACHE["nc"] = _build_bass()
    return _NC_CACHE["nc"]


def kernel(template, source, _trace=False):
    from concourse.bass_utils import run_bass_kernel_spmd

    template = np.asarray(template)
    source = np.asarray(source)
    assert template.shape == (B, NPTS, 3) and source.shape == (B, MPTS, 3)

    eye = np.eye(PTILE, dtype=BF)
    in_maps = []
    for core in range(NCORES):
        b, h = core // 2, core % 2
        ta, sa = _prep_core(template[b, h * NT : (h + 1) * NT], source[b])
        in_maps.append({"ta": ta, "sa": sa, "ident": eye})

    nc = _get_nc()
    res = run_bass_kernel_spmd(
        nc, in_maps, core_ids=list(range(NCORES)), trace=_trace
    )
    results = res.results

    out = np.zeros(B, dtype=np.float64)
    for b in range(B):
        r0, r1 = results[2 * b], results[2 * b + 1]
        d01 = (
            r0["rowmins"].astype(np.float64).sum()
            + r1["rowmins"].astype(np.float64).sum()
        ) / float(NPTS)
        c0 = r0["colmins"].T.reshape(-1)  # [MPTS], source idx = 128*k + p
        c1 = r1["colmins"].T.reshape(-1)
        d10 = np.minimum(c0, c1).astype(np.float64).mean()
        out[b] = d01 + d10
    if _trace:
        kernel._last_results = res
    return out.astype(np.float32)



# revision 28
# speedup vs baseline: 1.0109x; 1.0109x over previous
"""Chamfer distance kernel for 8 Trainium2 NeuronCores.

Problem: template [4, 8192, 3], source [4, 8192, 3] (fp32)
  d[b,n,m] = ||template[b,n] - source[b,m]||^2
  out[b] = mean_n min_m d + mean_m min_n d            (shape [4], fp32)

Sharding: 8 cores = 4 batches x 2 template-halves. Each core computes its
4096x8192 block of the distance matrix ONCE on the TensorEngine (augmented
K=18 matmul: d = n0 + n1 - 2<t,s>, with bf16 hi/lo coordinate splits so
every product is exact in fp32 PSUM accumulation), and reduces it in both
directions. DVE is the bottleneck at ~2 min-ALU-ops per element (one per
direction) at 2 ops/cycle in bf16 2x mode; everything else is arranged to
keep it gap-free:
  - ScalarE converts each PSUM tile to a bf16 SBUF row-panel (the only
    engine besides DVE that can read PSUM, and it cannot min).
  - col-mins: one wide DVE TT-min accumulate per row tile, partition-
    reduced at the end through PE transposes + DVE segmented reduces,
    with per-round output streaming.
  - row-mins: two-level TT-min halving per tile (8192->2048 into a
    rowtail slot), then batched 4-slot group folds + one 1x reduce per
    4 tiles (minimizes per-op init overhead).
  - ramp: row tiles 0-1 are processed chunk-by-chunk behind the input
    DMA; tiles <=2 run on PE row-group 0 only while the on-chip DMA
    builds the partition-offset-32 input replica that later tiles use
    for LDWEIGHTS/matmul row-group alternation.
Host combines: d01 from row-min sums, d10 from elementwise min of the two
halves' col-min vectors.
"""

import numpy as np
import ml_dtypes

BF = ml_dtypes.bfloat16

B = 4
NPTS = 8192  # template points per batch
MPTS = 8192  # source points per batch
NCORES = 8
NT = NPTS // 2  # template rows per core (half batch)
K = 18  # augmented contraction slots
PTILE = 128  # row tile (PSUM partitions)
CW = 2048  # ScalarE copy width (4 PSUM banks per psum tile)
NCP = MPTS // CW  # 2 copies per row tile
NROW = NT // PTILE  # 32 row tiles
NCOLK = MPTS // PTILE  # 64 columns of colmins output
HALVE_STOP = 2048  # per-tile chain stops here; group folds + reduce finish

_BIG = 3.0e38


def _bf16_parts(x64, n):
    """Split float64 array into n bf16 terms; sum of terms ~= x64."""
    parts = []
    r = np.array(x64, dtype=np.float64, copy=True)
    for _ in range(n):
        p = r.astype(BF)
        parts.append(p)
        r -= p.astype(np.float64)
    return parts


def _prep_core(templ_half, source):
    """Build the [K, NT] and [K, MPTS] bf16 slot matrices for one core.

    Slot layout (template side . source side):
      per coord c: (xh, xh, xl, xl) . (-2yh, -2yl, -2yh, -2yl)   -> 12 slots
      n0 (3-way split) . (1, 1, 1)                                -> 3 slots
      (1, 1, 1) . n1 (3-way split)                                -> 3 slots
    so sum_k ta[k,n]*sa[k,m] = ||t~_n - s~_m||^2 (t~, s~ = 16-bit-split
    coordinates; all bf16 products are exact in fp32 accumulation).
    """
    nt = templ_half.shape[0]
    ms = source.shape[0]
    t = templ_half.astype(np.float64)
    s = source.astype(np.float64)
    ta = np.zeros((K, nt), dtype=BF)
    sa = np.zeros((K, ms), dtype=BF)
    t_eff = np.zeros_like(t)
    s_eff = np.zeros_like(s)
    k = 0
    for c in range(3):
        xh, xl = _bf16_parts(t[:, c], 2)
        yh, yl = _bf16_parts(s[:, c], 2)
        t_eff[:, c] = xh.astype(np.float64) + xl.astype(np.float64)
        s_eff[:, c] = yh.astype(np.float64) + yl.astype(np.float64)
        m2yh = (-2.0 * yh.astype(np.float64)).astype(BF)  # exact (x2 = exp+1)
        m2yl = (-2.0 * yl.astype(np.float64)).astype(BF)
        ta[k + 0], sa[k + 0] = xh, m2yh
        ta[k + 1], sa[k + 1] = xh, m2yl
        ta[k + 2], sa[k + 2] = xl, m2yh
        ta[k + 3], sa[k + 3] = xl, m2yl
        k += 4
    n0 = (t_eff**2).sum(axis=1)
    n1 = (s_eff**2).sum(axis=1)
    ones_t = np.ones(nt, dtype=BF)
    ones_s = np.ones(ms, dtype=BF)
    for part in _bf16_parts(n0, 3):
        ta[k], sa[k] = part, ones_s
        k += 1
    for part in _bf16_parts(n1, 3):
        ta[k], sa[k] = ones_t, part
        k += 1
    assert k == K
    return ta, sa


def _build_bass():
    from contextlib import ExitStack

    import concourse.bacc as bacc
    import concourse.tile as tile
    from concourse import mybir

    f32 = mybir.dt.float32
    bf16 = mybir.dt.bfloat16
    MIN = mybir.AluOpType.min

    nc = bacc.Bacc("TRN2", target_bir_lowering=False)
    ta = nc.dram_tensor("ta", [K, NT], bf16, kind="ExternalInput")
    sa = nc.dram_tensor("sa", [K, MPTS], bf16, kind="ExternalInput")
    ident = nc.dram_tensor("ident", [PTILE, PTILE], bf16, kind="ExternalInput")
    rowmins = nc.dram_tensor("rowmins", [PTILE, NROW], f32, kind="ExternalOutput")
    colmins = nc.dram_tensor("colmins", [PTILE, NCOLK], f32, kind="ExternalOutput")

    with tile.TileContext(nc) as tc, ExitStack() as ctx:
        consts = ctx.enter_context(tc.tile_pool(name="consts", bufs=1))
        accs = ctx.enter_context(tc.tile_pool(name="accs", bufs=1))
        dpool = ctx.enter_context(tc.tile_pool(name="dpool", bufs=4))
        pspool = ctx.enter_context(tc.tile_pool(name="ps", bufs=2, space="PSUM"))

        # Input loads, ordered so row-tile 0 can start as early as possible:
        # tile-0 weights (tiny) first, then the sa column chunks (split
        # between the two HWDGE queues), then the rest of ta. The replica at
        # partition offset 32 (for PE row-group alternation from tile 1 on)
        # is made by on-chip SBUF->SBUF DMA instead of a second HBM load.
        ta_s = consts.tile([32 + K, NT], bf16, name="ta_s", tag="ta_s")
        sa_s = consts.tile([32 + K, MPTS], bf16, name="sa_s", tag="sa_s")
        nc.sync.dma_start(out=ta_s[0:K, 0:PTILE], in_=ta[:, 0:PTILE])
        for c in range(4):
            lsl = slice(c * 2048, c * 2048 + 1024)
            rsl = slice(c * 2048 + 1024, (c + 1) * 2048)
            nc.sync.dma_start(out=sa_s[0:K, lsl], in_=sa[:, lsl])
            nc.scalar.dma_start(out=sa_s[0:K, rsl], in_=sa[:, rsl])
        nc.scalar.dma_start(out=ta_s[0:K, PTILE:], in_=ta[:, PTILE:])
        for c in range(4):
            csl = slice(c * 2048, (c + 1) * 2048)
            nc.gpsimd.dma_start(out=sa_s[32 : 32 + K, csl], in_=sa_s[0:K, csl])
        nc.gpsimd.dma_start(out=ta_s[32 : 32 + K, :], in_=ta_s[0:K, :])
        id_s = consts.tile([PTILE, PTILE], bf16, name="id_s", tag="id_s")
        nc.gpsimd.dma_start(out=id_s, in_=ident[:, :])

        colacc = accs.tile([PTILE, MPTS], bf16, name="colacc", tag="colacc")
        rowtail = accs.tile(
            [PTILE, 4, HALVE_STOP], bf16, name="rowtail", tag="rowtail"
        )
        rowmins_s = accs.tile([PTILE, NROW], f32, name="rowmins_s", tag="rowmins_s")
        colmins_s = accs.tile([PTILE, NCOLK], f32, name="colmins_s", tag="colmins_s")

        def emit_tile_matmuls(ti, d, base):
            """Matmuls + ScalarE copies for row tile ti into d[:, base:base+MPTS]."""
            for cp in range(NCP):
                ps = pspool.tile([PTILE, CW], f32, name="ps", tag="ps")
                for q in range(CW // 512):
                    col0 = cp * CW + q * 512
                    rg = 0 if ti <= 2 else 32 * ((cp * (CW // 512) + q) % 2)
                    nc.tensor.matmul(
                        ps[:, q * 512 : (q + 1) * 512],
                        ta_s[rg : rg + K, ti * PTILE : (ti + 1) * PTILE],
                        sa_s[rg : rg + K, col0 : col0 + 512],
                        start=True,
                        stop=True,
                        tile_position=(rg, 0),
                    )
                dsl = slice(base + cp * CW, base + (cp + 1) * CW)
                nc.scalar.copy(d[:, dsl], ps)
                if ti <= 1:
                    # Tiles 0 and 1 are processed chunk-by-chunk so DVE work
                    # starts as soon as each SE chunk copy lands — this hides
                    # the input-DMA + ScalarE pipeline ramp. Chunk row-mins
                    # accumulate into the tile's rowtail slot.
                    if ti == 0:
                        nc.vector.tensor_copy(colacc[:, dsl], d[:, dsl])
                    else:
                        nc.vector.tensor_tensor(
                            out=colacc[:, dsl],
                            in0=d[:, dsl],
                            in1=colacc[:, dsl],
                            op=MIN,
                        )
                    if cp == 0:
                        nc.vector.tensor_copy(rowtail[:, ti, :], d[:, dsl])
                    else:
                        nc.vector.tensor_tensor(
                            out=rowtail[:, ti, :],
                            in0=d[:, dsl],
                            in1=rowtail[:, ti, :],
                            op=MIN,
                        )

        def emit_group_reduce(hi):
            """Fold rowtail slots 0..3 (tiles hi-3..hi) into rowmins."""
            w = HALVE_STOP // 2
            while w >= 128:
                nc.vector.tensor_tensor(
                    out=rowtail[:, :, 0:w],
                    in0=rowtail[:, :, 0:w],
                    in1=rowtail[:, :, w : 2 * w],
                    op=MIN,
                )
                w //= 2
            nc.vector.tensor_reduce(
                out=rowmins_s[:, hi - 3 : hi + 1],
                in_=rowtail[:, :, 0:128],
                axis=mybir.AxisListType.X,
                op=MIN,
            )

        # Tiles 0-3 run singly (tiles 0-1 chunk-interleaved for the ramp);
        # tiles 4..31 run as 14 double-tiles: one [128, 2*MPTS] panel holds
        # two row tiles so fold1 and the rowtail fold are each ONE wide DVE
        # op per pair, cutting per-instruction init overhead.
        for i in range(4):
            d = dpool.tile([PTILE, MPTS], bf16, name="d", tag="d")
            emit_tile_matmuls(i, d, 0)
            if i <= 1:
                continue
            nc.vector.tensor_tensor(out=colacc, in0=d, in1=colacc, op=MIN)
            dv = d.rearrange("p (n c) -> p n c", c=CW)
            nc.vector.tensor_tensor(
                out=dv[:, 0::2, :], in0=dv[:, 0::2, :], in1=dv[:, 1::2, :], op=MIN
            )
            nc.vector.tensor_tensor(
                out=rowtail[:, i, :],
                in0=d[:, 0:CW],
                in1=d[:, 2 * CW : 3 * CW],
                op=MIN,
            )
            if i == 3:
                emit_group_reduce(3)

        for i in range(4, NROW, 2):
            d = dpool.tile([PTILE, 2 * MPTS], bf16, name="d", tag="d")
            emit_tile_matmuls(i, d, 0)
            emit_tile_matmuls(i + 1, d, MPTS)
            # Column accumulates must read the original panels, so they go
            # before the in-place row folds (same engine => program order).
            nc.vector.tensor_tensor(
                out=colacc, in0=d[:, 0:MPTS], in1=colacc, op=MIN
            )
            nc.vector.tensor_tensor(
                out=colacc, in0=d[:, MPTS : 2 * MPTS], in1=colacc, op=MIN
            )
            # Row folds for BOTH tiles in single wide 3D-AP ops.
            dv = d.rearrange("p (n c) -> p n c", c=CW)  # [P, 8, CW]
            nc.vector.tensor_tensor(
                out=dv[:, 0::2, :], in0=dv[:, 0::2, :], in1=dv[:, 1::2, :], op=MIN
            )
            s0 = i % 4
            nc.vector.tensor_tensor(
                out=rowtail[:, s0 : s0 + 2, :],
                in0=dv[:, 0::4, :],
                in1=dv[:, 2::4, :],
                op=MIN,
            )
            if i % 4 == 2:
                emit_group_reduce(i + 1)

        # Row mins are complete after the last group reduce — store them
        # while the endgame runs.
        nc.sync.dma_start(out=rowmins[:, :], in_=rowmins_s)

        # Partition-reduce the column accumulators: PE transpose 128x128
        # blocks into PSUM (as bf16 slices of the fp32 pool tiles, one per
        # 2KB bank), then DVE segmented min-reduce (3D AP, axis X).
        kk = 0
        nper = CW // 512  # transposes per psum tile (one per bank)
        for t0 in range(0, NCOLK, nper):
            ps = pspool.tile([PTILE, CW], f32, name="ps", tag="ps")
            psb = ps.bitcast(bf16)  # [128, 2*CW] bf16 view
            for u in range(nper):
                t = t0 + u  # source block index: points 128*t .. 128*t+127
                nc.tensor.transpose(
                    psb[:, u * 1024 : u * 1024 + PTILE],
                    colacc[:, t * PTILE : (t + 1) * PTILE],
                    id_s,
                )
            seg = psb.rearrange("p (n x) -> p n x", x=1024)[:, :, :PTILE]
            nc.vector.tensor_reduce(
                out=colmins_s[:, kk : kk + nper],
                in_=seg,
                axis=mybir.AxisListType.X,
                op=MIN,
            )
            # Stream this round's colmins out immediately so only the last
            # round's store sits on the tail.
            nc.sync.dma_start(
                out=colmins[:, kk : kk + nper], in_=colmins_s[:, kk : kk + nper]
            )
            kk += nper
        assert kk == NCOLK
    nc.compile()
    return nc


_NC_CACHE = {}


def _get_nc():
    if "nc" not in _NC_CACHE:
        _NC_CACHE["nc"] = _build_bass()
    return _NC_CACHE["nc"]


def kernel(template, source, _trace=False):
    from concourse.bass_utils import run_bass_kernel_spmd

    template = np.asarray(template)
    source = np.asarray(source)
    assert template.shape == (B, NPTS, 3) and source.shape == (B, MPTS, 3)

    eye = np.eye(PTILE, dtype=BF)
    in_maps = []
    for core in range(NCORES):
        b, h = core // 2, core % 2
        ta, sa = _prep_core(template[b, h * NT : (h + 1) * NT], source[b])
        in_maps.append({"ta": ta, "sa": sa, "ident": eye})

    nc = _get_nc()
    res = run_bass_kernel_spmd(
        nc, in_maps, core_ids=list(range(NCORES)), trace=_trace
    )
    results = res.results

    out = np.zeros(B, dtype=np.float64)
    for b in range(B):
        r0, r1 = results[2 * b], results[2 * b + 1]
        d01 = (
            r0["rowmins"].astype(np.float64).sum()
            + r1["rowmins"].astype(np.float64).sum()
        ) / float(NPTS)
        c0 = r0["colmins"].T.reshape(-1)  # [MPTS], source idx = 128*k + p
        c1 = r1["colmins"].T.reshape(-1)
        d10 = np.minimum(c0, c1).astype(np.float64).mean()
        out[b] = d01 + d10
    if _trace:
        kernel._last_results = res
    return out.astype(np.float32)



# revision 30
# speedup vs baseline: 1.0208x; 1.0098x over previous
"""Chamfer distance kernel for 8 Trainium2 NeuronCores.

Problem: template [4, 8192, 3], source [4, 8192, 3] (fp32)
  d[b,n,m] = ||template[b,n] - source[b,m]||^2
  out[b] = mean_n min_m d + mean_m min_n d            (shape [4], fp32)

Sharding: 8 cores = 4 batches x 2 template-halves. Each core computes its
4096x8192 block of the distance matrix ONCE on the TensorEngine (augmented
K=18 matmul: d = n0 + n1 - 2<t,s>, with bf16 hi/lo coordinate splits so
every product is exact in fp32 PSUM accumulation), and reduces it in both
directions. DVE is the bottleneck at ~2 min-ALU-ops per element (one per
direction) at 2 ops/cycle in bf16 2x mode; everything else is arranged to
keep it gap-free:
  - ScalarE converts each PSUM tile to a bf16 SBUF row-panel (the only
    engine besides DVE that can read PSUM, and it cannot min).
  - col-mins: one wide DVE TT-min accumulate per row tile, partition-
    reduced at the end through PE transposes + DVE segmented reduces,
    with per-round output streaming.
  - row-mins: two-level TT-min halving per tile (8192->2048 into a
    rowtail slot), then batched 4-slot group folds + one 1x reduce per
    4 tiles (minimizes per-op init overhead).
  - ramp: row tiles 0-1 are processed chunk-by-chunk behind the input
    DMA; tiles <=2 run on PE row-group 0 only while the on-chip DMA
    builds the partition-offset-32 input replica that later tiles use
    for LDWEIGHTS/matmul row-group alternation.
Host combines: d01 from row-min sums, d10 from elementwise min of the two
halves' col-min vectors.
"""

import numpy as np
import ml_dtypes

BF = ml_dtypes.bfloat16

B = 4
NPTS = 8192  # template points per batch
MPTS = 8192  # source points per batch
NCORES = 8
NT = NPTS // 2  # template rows per core (half batch)
K = 18  # augmented contraction slots
PTILE = 128  # row tile (PSUM partitions)
CW = 2048  # ScalarE copy width (4 PSUM banks per psum tile)
NCP = MPTS // CW  # 2 copies per row tile
NROW = NT // PTILE  # 32 row tiles
NCOLK = MPTS // PTILE  # 64 columns of colmins output
HALVE_STOP = 2048  # per-tile chain stops here; group folds + reduce finish

_BIG = 3.0e38


def _bf16_parts(x64, n):
    """Split float64 array into n bf16 terms; sum of terms ~= x64."""
    parts = []
    r = np.array(x64, dtype=np.float64, copy=True)
    for _ in range(n):
        p = r.astype(BF)
        parts.append(p)
        r -= p.astype(np.float64)
    return parts


def _prep_core(templ_half, source):
    """Build the [K, NT] and [K, MPTS] bf16 slot matrices for one core.

    Slot layout (template side . source side):
      per coord c: (xh, xh, xl, xl) . (-2yh, -2yl, -2yh, -2yl)   -> 12 slots
      n0 (3-way split) . (1, 1, 1)                                -> 3 slots
      (1, 1, 1) . n1 (3-way split)                                -> 3 slots
    so sum_k ta[k,n]*sa[k,m] = ||t~_n - s~_m||^2 (t~, s~ = 16-bit-split
    coordinates; all bf16 products are exact in fp32 accumulation).
    """
    nt = templ_half.shape[0]
    ms = source.shape[0]
    t = templ_half.astype(np.float64)
    s = source.astype(np.float64)
    ta = np.zeros((K, nt), dtype=BF)
    sa = np.zeros((K, ms), dtype=BF)
    t_eff = np.zeros_like(t)
    s_eff = np.zeros_like(s)
    k = 0
    for c in range(3):
        xh, xl = _bf16_parts(t[:, c], 2)
        yh, yl = _bf16_parts(s[:, c], 2)
        t_eff[:, c] = xh.astype(np.float64) + xl.astype(np.float64)
        s_eff[:, c] = yh.astype(np.float64) + yl.astype(np.float64)
        m2yh = (-2.0 * yh.astype(np.float64)).astype(BF)  # exact (x2 = exp+1)
        m2yl = (-2.0 * yl.astype(np.float64)).astype(BF)
        ta[k + 0], sa[k + 0] = xh, m2yh
        ta[k + 1], sa[k + 1] = xh, m2yl
        ta[k + 2], sa[k + 2] = xl, m2yh
        ta[k + 3], sa[k + 3] = xl, m2yl
        k += 4
    n0 = (t_eff**2).sum(axis=1)
    n1 = (s_eff**2).sum(axis=1)
    ones_t = np.ones(nt, dtype=BF)
    ones_s = np.ones(ms, dtype=BF)
    for part in _bf16_parts(n0, 3):
        ta[k], sa[k] = part, ones_s
        k += 1
    for part in _bf16_parts(n1, 3):
        ta[k], sa[k] = ones_t, part
        k += 1
    assert k == K
    return ta, sa


def _build_bass():
    from contextlib import ExitStack

    import concourse.bacc as bacc
    import concourse.tile as tile
    from concourse import mybir

    f32 = mybir.dt.float32
    bf16 = mybir.dt.bfloat16
    MIN = mybir.AluOpType.min

    nc = bacc.Bacc("TRN2", target_bir_lowering=False)
    ta = nc.dram_tensor("ta", [K, NT], bf16, kind="ExternalInput")
    sa = nc.dram_tensor("sa", [K, MPTS], bf16, kind="ExternalInput")
    ident = nc.dram_tensor("ident", [PTILE, PTILE], bf16, kind="ExternalInput")
    rowmins = nc.dram_tensor("rowmins", [PTILE, NROW], f32, kind="ExternalOutput")
    colmins = nc.dram_tensor("colmins", [PTILE, NCOLK], f32, kind="ExternalOutput")

    with tile.TileContext(nc) as tc, ExitStack() as ctx:
        consts = ctx.enter_context(tc.tile_pool(name="consts", bufs=1))
        accs = ctx.enter_context(tc.tile_pool(name="accs", bufs=1))
        dpool = ctx.enter_context(tc.tile_pool(name="dpool", bufs=4))
        pspool = ctx.enter_context(tc.tile_pool(name="ps", bufs=2, space="PSUM"))

        # Input loads, ordered so row-tile 0 can start as early as possible:
        # tile-0 weights (tiny) first, then the sa column chunks (split
        # between the two HWDGE queues), then the rest of ta. The replica at
        # partition offset 32 (for PE row-group alternation from tile 1 on)
        # is made by on-chip SBUF->SBUF DMA instead of a second HBM load.
        ta_s = consts.tile([32 + K, NT], bf16, name="ta_s", tag="ta_s")
        sa_s = consts.tile([32 + K, MPTS], bf16, name="sa_s", tag="sa_s")
        nc.sync.dma_start(out=ta_s[0:K, 0:PTILE], in_=ta[:, 0:PTILE])
        for c in range(4):
            lsl = slice(c * 2048, c * 2048 + 1024)
            rsl = slice(c * 2048 + 1024, (c + 1) * 2048)
            nc.sync.dma_start(out=sa_s[0:K, lsl], in_=sa[:, lsl])
            nc.scalar.dma_start(out=sa_s[0:K, rsl], in_=sa[:, rsl])
        nc.scalar.dma_start(out=ta_s[0:K, PTILE:], in_=ta[:, PTILE:])
        for c in range(4):
            csl = slice(c * 2048, (c + 1) * 2048)
            nc.gpsimd.dma_start(out=sa_s[32 : 32 + K, csl], in_=sa_s[0:K, csl])
        nc.gpsimd.dma_start(out=ta_s[32 : 32 + K, :], in_=ta_s[0:K, :])
        id_s = consts.tile([PTILE, PTILE], bf16, name="id_s", tag="id_s")
        nc.gpsimd.dma_start(out=id_s, in_=ident[:, :])

        colacc = accs.tile([PTILE, MPTS], bf16, name="colacc", tag="colacc")
        rowtail = accs.tile(
            [PTILE, 4, HALVE_STOP], bf16, name="rowtail", tag="rowtail"
        )
        rowmins_s = accs.tile([PTILE, NROW], f32, name="rowmins_s", tag="rowmins_s")
        colmins_s = accs.tile([PTILE, NCOLK], f32, name="colmins_s", tag="colmins_s")

        def emit_tile_matmuls(ti, d, base):
            """Matmuls + ScalarE copies for row tile ti into d[:, base:base+MPTS]."""
            for cp in range(NCP):
                ps = pspool.tile([PTILE, CW], f32, name="ps", tag="ps")
                for q in range(CW // 512):
                    col0 = cp * CW + q * 512
                    rg = 0 if ti <= 2 else 32 * ((cp * (CW // 512) + q) % 2)
                    nc.tensor.matmul(
                        ps[:, q * 512 : (q + 1) * 512],
                        ta_s[rg : rg + K, ti * PTILE : (ti + 1) * PTILE],
                        sa_s[rg : rg + K, col0 : col0 + 512],
                        start=True,
                        stop=True,
                        tile_position=(rg, 0),
                    )
                dsl = slice(base + cp * CW, base + (cp + 1) * CW)
                nc.scalar.copy(d[:, dsl], ps)
                if ti <= 3:
                    # Tiles 0-3 are processed chunk-by-chunk so DVE work
                    # starts as soon as each SE chunk copy lands — this hides
                    # the input-DMA + ScalarE pipeline ramp (ScalarE must
                    # emit three full panels before DVE's steady state).
                    # Chunk row-mins accumulate into the tile's rowtail slot.
                    if ti == 0:
                        nc.vector.tensor_copy(colacc[:, dsl], d[:, dsl])
                    else:
                        nc.vector.tensor_tensor(
                            out=colacc[:, dsl],
                            in0=d[:, dsl],
                            in1=colacc[:, dsl],
                            op=MIN,
                        )
                    if cp == 0:
                        nc.vector.tensor_copy(rowtail[:, ti, :], d[:, dsl])
                    else:
                        nc.vector.tensor_tensor(
                            out=rowtail[:, ti, :],
                            in0=d[:, dsl],
                            in1=rowtail[:, ti, :],
                            op=MIN,
                        )

        def emit_group_reduce(hi):
            """Fold rowtail slots 0..3 (tiles hi-3..hi) into rowmins."""
            w = HALVE_STOP // 2
            while w >= 128:
                nc.vector.tensor_tensor(
                    out=rowtail[:, :, 0:w],
                    in0=rowtail[:, :, 0:w],
                    in1=rowtail[:, :, w : 2 * w],
                    op=MIN,
                )
                w //= 2
            nc.vector.tensor_reduce(
                out=rowmins_s[:, hi - 3 : hi + 1],
                in_=rowtail[:, :, 0:128],
                axis=mybir.AxisListType.X,
                op=MIN,
            )

        # Tiles 0-3 run singly (tiles 0-1 chunk-interleaved for the ramp);
        # tiles 4..31 run as 14 double-tiles: one [128, 2*MPTS] panel holds
        # two row tiles so fold1 and the rowtail fold are each ONE wide DVE
        # op per pair, cutting per-instruction init overhead.
        for i in range(4):
            d = dpool.tile([PTILE, MPTS], bf16, name="d", tag="d")
            emit_tile_matmuls(i, d, 0)
            if i == 3:
                emit_group_reduce(3)

        for i in range(4, NROW, 2):
            d = dpool.tile([PTILE, 2 * MPTS], bf16, name="d", tag="d")
            emit_tile_matmuls(i, d, 0)
            emit_tile_matmuls(i + 1, d, MPTS)
            # Column accumulates must read the original panels, so they go
            # before the in-place row folds (same engine => program order).
            nc.vector.tensor_tensor(
                out=colacc, in0=d[:, 0:MPTS], in1=colacc, op=MIN
            )
            nc.vector.tensor_tensor(
                out=colacc, in0=d[:, MPTS : 2 * MPTS], in1=colacc, op=MIN
            )
            # Row folds for BOTH tiles in single wide 3D-AP ops.
            dv = d.rearrange("p (n c) -> p n c", c=CW)  # [P, 8, CW]
            nc.vector.tensor_tensor(
                out=dv[:, 0::2, :], in0=dv[:, 0::2, :], in1=dv[:, 1::2, :], op=MIN
            )
            s0 = i % 4
            nc.vector.tensor_tensor(
                out=rowtail[:, s0 : s0 + 2, :],
                in0=dv[:, 0::4, :],
                in1=dv[:, 2::4, :],
                op=MIN,
            )
            if i % 4 == 2:
                emit_group_reduce(i + 1)

        # Row mins are complete after the last group reduce — store them
        # while the endgame runs.
        nc.sync.dma_start(out=rowmins[:, :], in_=rowmins_s)

        # Partition-reduce the column accumulators: PE transpose 128x128
        # blocks into PSUM (as bf16 slices of the fp32 pool tiles, one per
        # 2KB bank), then DVE segmented min-reduce (3D AP, axis X).
        kk = 0
        nper = CW // 512  # transposes per psum tile (one per bank)
        for t0 in range(0, NCOLK, nper):
            ps = pspool.tile([PTILE, CW], f32, name="ps", tag="ps")
            psb = ps.bitcast(bf16)  # [128, 2*CW] bf16 view
            for u in range(nper):
                t = t0 + u  # source block index: points 128*t .. 128*t+127
                nc.tensor.transpose(
                    psb[:, u * 1024 : u * 1024 + PTILE],
                    colacc[:, t * PTILE : (t + 1) * PTILE],
                    id_s,
                )
            seg = psb.rearrange("p (n x) -> p n x", x=1024)[:, :, :PTILE]
            nc.vector.tensor_reduce(
                out=colmins_s[:, kk : kk + nper],
                in_=seg,
                axis=mybir.AxisListType.X,
                op=MIN,
            )
            # Stream this round's colmins out immediately so only the last
            # round's store sits on the tail.
            nc.sync.dma_start(
                out=colmins[:, kk : kk + nper], in_=colmins_s[:, kk : kk + nper]
            )
            kk += nper
        assert kk == NCOLK
    nc.compile()
    return nc


_NC_CACHE = {}


def _get_nc():
    if "nc" not in _NC_CACHE:
        _NC_CACHE["nc"] = _build_bass()
    return _NC_CACHE["nc"]


def kernel(template, source, _trace=False):
    from concourse.bass_utils import run_bass_kernel_spmd

    template = np.asarray(template)
    source = np.asarray(source)
    assert template.shape == (B, NPTS, 3) and source.shape == (B, MPTS, 3)

    eye = np.eye(PTILE, dtype=BF)
    in_maps = []
    for core in range(NCORES):
        b, h = core // 2, core % 2
        ta, sa = _prep_core(template[b, h * NT : (h + 1) * NT], source[b])
        in_maps.append({"ta": ta, "sa": sa, "ident": eye})

    nc = _get_nc()
    res = run_bass_kernel_spmd(
        nc, in_maps, core_ids=list(range(NCORES)), trace=_trace
    )
    results = res.results

    out = np.zeros(B, dtype=np.float64)
    for b in range(B):
        r0, r1 = results[2 * b], results[2 * b + 1]
        d01 = (
            r0["rowmins"].astype(np.float64).sum()
            + r1["rowmins"].astype(np.float64).sum()
        ) / float(NPTS)
        c0 = r0["colmins"].T.reshape(-1)  # [MPTS], source idx = 128*k + p
        c1 = r1["colmins"].T.reshape(-1)
        d10 = np.minimum(c0, c1).astype(np.float64).mean()
        out[b] = d01 + d10
    if _trace:
        kernel._last_results = res
    return out.astype(np.float32)

